# revision 3
# baseline (speedup 1.0000x reference)
"""GraphSAGE-mean 2-layer GNN on 8 Trainium2 NeuronCores (Bass/Tile).

Sharding: nodes split into 8 contiguous ranges (rows c*12500..): core c
computes output rows for its range.  The full feature table is replicated per
core; layer-1 results are AllGather'd to rebuild the replicated table for
layer 2.

Aggregation: per core, edges (grouped by dst) are split into 4 passes by src
chunk of 32768 rows so src indices fit the int16 index format of the custom
dma_gather ucode (4096 rows per instruction).  Segment-sum runs on the tensor
engine: for each 128-edge block a selection matrix
  sel[e, m] = (dstl[e] == m) * invdeg[dst[e]]
is built in one fused DVE op from a constant iota tile, and
  psum[f, m] += msgs[e, f]^T @ sel[e, m]
accumulates weighted neighbor sums for one 128-node tile, feature-major.
The self path is contiguous loads + PE transpose; the transform computes
out^T = W_neigh^T @ aggT + W_self^T @ selfT with bias+relu fused into one
ScalarE activation, then PE-transposes back to node-major rows.

The SPMD program is shared by all 8 cores, so per-(pass, tile) block counts
are static = max over the 8 cores; shorter cores pad with zero-weight slots.
"""

import numpy as np

N = 100000
F = 128
NCORES = 8
OWN = N // NCORES            # 12500
P = 128
NTILES = (OWN + P - 1) // P  # 98
OWN_PAD = NTILES * P         # 12544
N_PAD = 100096               # table rows padded to a multiple of 128
CHUNK = 32768
NPASS = (N + CHUNK - 1) // CHUNK  # 4
GBS = 1024                   # gather rows per dma_gather instruction (SWDGE ring holds 1024 descs)
BLK = 128                    # edges per block


# --------------------------------------------------------------------------
# host-side planning
# --------------------------------------------------------------------------

def _plan(edge_src, edge_dst):
    src = np.asarray(edge_src).astype(np.int64).ravel()
    dst = np.asarray(edge_dst).astype(np.int64).ravel()
    deg = np.bincount(dst, minlength=N)
    invdeg = (1.0 / np.maximum(deg, 1)).astype(np.float32)

    per_core = []
    owner = dst // OWN
    for c in range(NCORES):
        m = owner == c
        s, d = src[m], dst[m]
        p = s // CHUNK
        order = np.lexsort((d, p))
        per_core.append((s[order], d[order], p[order]))

    cnt = np.zeros((NCORES, NPASS, NTILES), dtype=np.int64)
    for c in range(NCORES):
        s, d, p = per_core[c]
        t = (d - c * OWN) // P
        np.add.at(cnt, (c, p, t), 1)
    B = np.ceil(cnt.max(axis=0) / BLK).astype(np.int64)   # [NPASS, NTILES]

    nblk_pass = B.sum(axis=1).astype(np.int64)
    nblk = int(nblk_pass.sum())
    blk_tile = np.concatenate(
        [np.repeat(np.arange(NTILES), B[p]) for p in range(NPASS)]
    ).astype(np.int64)

    plans = []
    for c in range(NCORES):
        s, d, p = per_core[c]
        idx16 = np.zeros(nblk * BLK, dtype=np.int16)
        dstl = np.full(nblk * BLK, -1.0, dtype=np.float32)
        w = np.zeros(nblk * BLK, dtype=np.float32)
        blk0 = 0
        for pp in range(NPASS):
            m = p == pp
            sp, dp = s[m], d[m]
            tp = (dp - c * OWN) // P
            for t in range(NTILES):
                bcount = int(B[pp, t])
                if bcount == 0:
                    continue
                em = tp == t
                se, de = sp[em], dp[em]
                ne = se.shape[0]
                assert ne <= bcount * BLK
                base = blk0 * BLK
                idx16[base : base + ne] = (se - pp * CHUNK).astype(np.int16)
                dstl[base : base + ne] = (de - c * OWN - t * P).astype(np.float32)
                w[base : base + ne] = invdeg[de]
                blk0 += bcount
        assert blk0 == nblk
        plans.append({"idx16": idx16, "dstl": dstl, "w": w})

    return plans, B, blk_tile, nblk_pass, nblk


def _gather_instruction_sizes(nblk_pass):
    """Mirror of the device loop: list of (pass, blocks) per gather inst."""
    out = []
    for pp in range(NPASS):
        nb = int(nblk_pass[pp])
        done = 0
        while done < nb:
            take = min(GBS // BLK, nb - done)
            out.append((pp, take))
            done += take
    return out


def _pack_gidx(idx16, nblk_pass):
    """Pack int16 indices in the dma_gather SBUF layout (position j ->
    partition j%16, column j//16, replicated to 128 partitions) as one
    [128, total_cols] plane with per-instruction column segments, raveled
    partition-major.  Loaded to SBUF once and sliced per instruction."""
    total_cols = sum(take * BLK // 16
                     for _pp, take in _gather_instruction_sizes(nblk_pass))
    out = np.zeros((128, total_cols), dtype=np.int16)
    cursor = 0
    col = 0
    for _pp, take in _gather_instruction_sizes(nblk_pass):
        rows = take * BLK
        seg = idx16[cursor : cursor + rows]
        cursor += rows
        w16 = seg.reshape(rows // 16, 16).T          # [16, cols]
        out[:, col : col + rows // 16] = np.tile(w16, (8, 1))
        col += rows // 16
    return out.ravel()


def _schedule_flags(B):
    """start/stop flags per block within each pass (blocks are emitted
    pass-major, grouped by tile)."""
    firsts, lasts = [], []
    for pp in range(NPASS):
        tiles = [int(t) for t in np.repeat(np.arange(NTILES), B[pp])]
        f = [i == 0 or tiles[i] != tiles[i - 1] for i in range(len(tiles))]
        l = [i + 1 == len(tiles) or tiles[i + 1] != tiles[i]
             for i in range(len(tiles))]
        firsts.append(f)
        lasts.append(l)
    return firsts, lasts


# --------------------------------------------------------------------------
# device program
# --------------------------------------------------------------------------

def _build(B, blk_tile, nblk_pass, nblk, skip_collective=False):
    import concourse.bass as bass
    import concourse.mybir as mybir
    import concourse.tile as tile
    from concourse import library_config
    from concourse.masks import make_identity
    from concourse.tile_rust import add_dep_helper

    nc = bass.Bass("TRN2", target_bir_lowering=False, debug=False,
                   num_devices=NCORES, num_swdge_queues=4)
    dt = mybir.dt

    x_rep = nc.dram_tensor("x_rep", [N_PAD, F], dt.float32, kind="ExternalInput")
    x_self = nc.dram_tensor("x_self", [OWN_PAD, F], dt.float32,
                            kind="ExternalInput")
    gidx_len = sum(128 * (take * BLK // 16)
                   for _pp, take in _gather_instruction_sizes(nblk_pass))
    gidx = nc.dram_tensor("gidx", [gidx_len], dt.int16, kind="ExternalInput")
    dstl_in = nc.dram_tensor("dstl", [P * nblk], dt.float32, kind="ExternalInput")
    w_in = nc.dram_tensor("w", [P * nblk], dt.float32, kind="ExternalInput")
    iota_in = nc.dram_tensor("iota", [P * P], dt.float32, kind="ExternalInput")
    ws1 = nc.dram_tensor("W_self1", [F, F], dt.float32, kind="ExternalInput")
    wn1 = nc.dram_tensor("W_neigh1", [F, F], dt.float32, kind="ExternalInput")
    b1 = nc.dram_tensor("b1", [F], dt.float32, kind="ExternalInput")
    ws2 = nc.dram_tensor("W_self2", [F, F], dt.float32, kind="ExternalInput")
    wn2 = nc.dram_tensor("W_neigh2", [F, F], dt.float32, kind="ExternalInput")
    b2 = nc.dram_tensor("b2", [F], dt.float32, kind="ExternalInput")
    out_shard = nc.dram_tensor("out_shard", [OWN_PAD, F], dt.float32,
                               kind="ExternalOutput")

    h1_own = nc.dram_tensor("h1_own", [OWN_PAD, F], dt.float32)
    h1_rep = nc.dram_tensor("h1_rep", [N_PAD, F], dt.float32,
                            addr_space="Shared")

    pass_len = [min(CHUNK, N - p * CHUNK) for p in range(NPASS)]
    firsts, lasts = _schedule_flags(B)
    inst_sizes = _gather_instruction_sizes(nblk_pass)

    with tile.TileContext(nc) as tc:
        with (
            tc.tile_pool(name="const", bufs=1) as cpool,
            tc.tile_pool(name="gather", bufs=6) as gpool,
            tc.tile_pool(name="sel", bufs=6) as spool,
            tc.tile_pool(name="acc", bufs=1) as apool,
            tc.tile_pool(name="stage", bufs=3) as stpool,
            tc.tile_pool(name="psA", bufs=2, space="PSUM") as ppoolA,
            tc.tile_pool(name="psB", bufs=2, space="PSUM") as ppoolB,
        ):
            lib = nc.gpsimd.load_library(library_config.mlp)
            rows_regs = {}

            def rows_reg(v):
                if v not in rows_regs:
                    rows_regs[v] = nc.gpsimd.to_reg(v)
                return rows_regs[v]

            iota = cpool.tile([P, P], dt.float32)
            nc.sync.dma_start(out=iota[:],
                              in_=iota_in.ap().rearrange("(p f) -> p f", p=P))
            ident = cpool.tile([P, P], dt.float32)
            make_identity(nc, ident[:])
            ident_bf = cpool.tile([P, P], dt.bfloat16)
            nc.vector.tensor_copy(out=ident_bf[:], in_=ident[:])

            wtiles = {}
            for name, t in (("ws1", ws1), ("wn1", wn1), ("ws2", ws2),
                            ("wn2", wn2)):
                wt = cpool.tile([P, P], dt.float32, name=f"w_{name}", tag=f"w_{name}")
                nc.sync.dma_start(out=wt[:], in_=t[:, :])
                wtiles[name] = wt
            btiles = {}
            for name, t in (("b1", b1), ("b2", b2)):
                bt = cpool.tile([P, 1], dt.float32, name=f"b_{name}", tag=f"b_{name}")
                nc.sync.dma_start(out=bt[:], in_=t.ap()[:, None])
                btiles[name] = bt

            gidx_t = cpool.tile([P, gidx_len // P], dt.int16)
            nc.sync.dma_start(out=gidx_t[:],
                              in_=gidx.ap().rearrange("(p k) -> p k", p=P))
            dstl_t = cpool.tile([P, nblk], dt.float32)
            nc.sync.dma_start(out=dstl_t[:],
                              in_=dstl_in.ap().rearrange("(p b) -> p b", p=P))
            w_t = cpool.tile([P, nblk], dt.float32)
            nc.sync.dma_start(out=w_t[:],
                              in_=w_in.ap().rearrange("(p b) -> p b", p=P))

            aggT = apool.tile([P, NTILES * P], dt.float32)
            selfT = apool.tile([P, NTILES * P], dt.float32)

            def run_layer(table, self_table, wself, wneigh, bias,
                          dest, tdt, ddt, identt):
                nc.vector.memset(aggT[:], 0.0)

                live_psum = {}
                blk_cursor = 0      # global block index
                gcol = 0            # idx columns consumed in gidx_t
                pass_blk = 0        # block index within current pass
                cur_pass = 0
                ginst = 0           # gather instruction counter
                for pp, take in inst_sizes:
                    if pp != cur_pass:
                        cur_pass = pp
                        pass_blk = 0
                    rows = take * BLK
                    icols = rows // 16
                    gt = gpool.tile([P, (GBS // BLK) * P], tdt, tag="g")
                    g = nc.gpsimd.dma_gather(
                        gt[:, : take * P].rearrange("p (b f) -> p b f", f=P),
                        table[pp * CHUNK : pp * CHUNK + pass_len[pp], :],
                        gidx_t[:, gcol : gcol + icols],
                        rows,
                        rows_reg(rows),
                        F,
                        queue_num=ginst % 4,
                    )
                    gcol += icols
                    ginst += 1
                    add_dep_helper(g.ins, lib.ins, sync=False,
                                   reason="ucode lib before gather")

                    for k in range(take):
                        b = blk_cursor + k
                        t = int(blk_tile[b])
                        sel = spool.tile([P, P], tdt, tag="sel")
                        nc.vector.tensor_scalar(
                            sel[:], iota[:],
                            dstl_t[:, b : b + 1], w_t[:, b : b + 1],
                            mybir.AluOpType.is_equal, mybir.AluOpType.mult,
                        )
                        if firsts[pp][pass_blk + k]:
                            live_psum[t] = ppoolA.tile([P, P], dt.float32, name="ps",
                                                       tag="ps", space="PSUM")
                        ps = live_psum[t]
                        nc.tensor.matmul(
                            out=ps[:], lhsT=gt[:, k * P : (k + 1) * P],
                            rhs=sel[:],
                            start=bool(firsts[pp][pass_blk + k]),
                            stop=bool(lasts[pp][pass_blk + k]),
                        )
                        if lasts[pp][pass_blk + k]:
                            nc.vector.tensor_tensor(
                                out=aggT[:, t * P : (t + 1) * P],
                                in0=aggT[:, t * P : (t + 1) * P],
                                in1=ps[:], op=mybir.AluOpType.add,
                            )
                            del live_psum[t]
                    blk_cursor += take
                    pass_blk += take

                for t in range(NTILES):
                    xt = stpool.tile([P, P], tdt, tag="xt")
                    nc.sync.dma_start(out=xt[:],
                                      in_=self_table[t * P : (t + 1) * P, :])
                    pst = ppoolB.tile([P, P], tdt, tag="pst", space="PSUM")
                    nc.tensor.transpose(out=pst[:], in_=xt[:],
                                        identity=identt[:])
                    nc.vector.tensor_copy(out=selfT[:, t * P : (t + 1) * P],
                                          in_=pst[:])

                writes = []
                for g0 in range(0, NTILES, 4):
                    tn = min(4, NTILES - g0)
                    wdt = tn * P
                    psT = ppoolB.tile([P, 512], dt.float32, tag="psT",
                                      space="PSUM")
                    nc.tensor.matmul(out=psT[:, :wdt], lhsT=wneigh[:],
                                     rhs=aggT[:, g0 * P : g0 * P + wdt],
                                     start=True, stop=False)
                    nc.tensor.matmul(out=psT[:, :wdt], lhsT=wself[:],
                                     rhs=selfT[:, g0 * P : g0 * P + wdt],
                                     start=False, stop=True)
                    oT = stpool.tile([P, 512], dt.float32, tag="oT")
                    nc.scalar.activation(oT[:, :wdt], psT[:, :wdt],
                                         mybir.ActivationFunctionType.Relu,
                                         bias=bias[:, :1])
                    ost = stpool.tile([P, 512], ddt, tag="ost")
                    for j in range(tn):
                        psX = ppoolA.tile([P, P], dt.float32, tag="psX",
                                          space="PSUM")
                        nc.tensor.transpose(out=psX[:],
                                            in_=oT[:, j * P : (j + 1) * P],
                                            identity=ident[:])
                        nc.vector.tensor_copy(
                            out=ost[:, j * P : (j + 1) * P], in_=psX[:])
                    dd = nc.sync.dma_start(
                        out=dest[g0 * P : g0 * P + wdt, :]
                        .rearrange("(j p) f -> p j f", p=P),
                        in_=ost[:, :wdt].rearrange("p (j f) -> p j f", f=P),
                    )
                    writes.append(dd)
                return writes

            run_layer(x_rep, x_self, wtiles["ws1"], wtiles["wn1"],
                      btiles["b1"], h1_own, dt.float32, dt.float32, ident)

            if skip_collective:
                nc.sync.dma_start(out=h1_rep[0:OWN, :], in_=h1_own[0:OWN, :])
            else:
                nc.gpsimd.collective_compute(
                    "AllGather",
                    mybir.AluOpType.bypass,
                    replica_groups=[list(range(NCORES))],
                    ins=[h1_own[0:OWN, :]],
                    outs=[h1_rep[0:N, :]],
                )
            if N_PAD > N:
                zt = stpool.tile([P, F], dt.float32, tag="zt")
                nc.vector.memset(zt[:], 0.0)
                nc.sync.dma_start(out=h1_rep[N:N_PAD, :],
                                  in_=zt[: N_PAD - N, :])

            run_layer(h1_rep, h1_own, wtiles["ws2"], wtiles["wn2"],
                      btiles["b2"], out_shard, dt.float32, dt.float32, ident)

    _split_multi_waits(nc)
    from concourse.library_overlay import lower_extended_insts
    lower_extended_insts(nc)
    return nc


def _split_multi_waits(nc):
    pass_impl = True
    """Walrus codegen encodes at most one sync wait per instruction; split
    extras into standalone EventSemaphore instructions on the same in-order
    engine queue (semantically identical)."""
    import concourse.mybir as mybir

    n = 0
    for f in nc.m.functions:
        for b in f.blocks:
            insts = b.instructions
            new_list = []
            for inst in insts:
                si = inst.sync_info
                if si is not None and len(si.on_wait) > 1:
                    waits = list(si.on_wait)
                    for wt in waits[:-1]:
                        ev = mybir.InstEventSemaphore(
                            name=f"evsplit-{n}",
                            engine=inst.engine,
                            sync_info=mybir.SyncInfo(on_wait=[wt],
                                                     on_update=[]),
                            ins=[], outs=[],
                        )
                        new_list.append(ev)
                        try:
                            nc.inst_map[ev.name] = ev
                        except Exception:
                            pass
                        n += 1
                    inst.sync_info = mybir.SyncInfo(
                        on_wait=[waits[-1]], on_update=list(si.on_update)
                    )
                new_list.append(inst)
            insts[:] = new_list
    return n


# --------------------------------------------------------------------------
# entry point
# --------------------------------------------------------------------------

def prepare(x, edge_src, edge_dst, W_self1, W_neigh1, b1, W_self2, W_neigh2,
            b2):
    """Plan + build + pack inputs; returns (nc, in_maps, finish)."""
    x = np.asarray(x, dtype=np.float32)
    plans, B, blk_tile, nblk_pass, nblk = _plan(edge_src, edge_dst)

    xpad = np.zeros((N_PAD, F), dtype=np.float32)
    xpad[:N] = x
    iota = np.broadcast_to(np.arange(P, dtype=np.float32), (P, P))

    in_maps = []
    for c in range(NCORES):
        pl = plans[c]
        xs = np.zeros((OWN_PAD, F), dtype=np.float32)
        xs[:OWN] = x[c * OWN : (c + 1) * OWN]
        in_maps.append({
            "x_rep": xpad,
            "x_self": xs,
            "gidx": _pack_gidx(pl["idx16"], nblk_pass),
            "dstl": pl["dstl"].reshape(nblk, P).T.copy().ravel(),
            "w": pl["w"].reshape(nblk, P).T.copy().ravel(),
            "iota": np.ascontiguousarray(iota).ravel(),
            "W_self1": np.asarray(W_self1, np.float32),
            "W_neigh1": np.asarray(W_neigh1, np.float32),
            "b1": np.asarray(b1, np.float32),
            "W_self2": np.asarray(W_self2, np.float32),
            "W_neigh2": np.asarray(W_neigh2, np.float32),
            "b2": np.asarray(b2, np.float32),
        })

    nc = _build(B, blk_tile, nblk_pass, nblk)

    def finish(results):
        return np.concatenate(
            [results[c]["out_shard"][:OWN] for c in range(NCORES)], axis=0
        ).astype(np.float32)

    return nc, in_maps, finish


def kernel(x, edge_src, edge_dst, W_self1, W_neigh1, b1, W_self2, W_neigh2,
           b2, trace=False, _return_res=False):
    from concourse.bass_utils import run_bass_kernel_spmd

    nc, in_maps, finish = prepare(x, edge_src, edge_dst, W_self1, W_neigh1,
                                  b1, W_self2, W_neigh2, b2)
    res = run_bass_kernel_spmd(nc, in_maps, list(range(NCORES)), trace=trace)
    out = finish(res.results)
    if _return_res:
        return out, res
    return out



# revision 34
# speedup vs baseline: 32.3794x; 32.3794x over previous
"""GraphSAGE-mean 2-layer GNN on 8 Trainium2 NeuronCores (Bass/Tile).

Sharding: nodes split into 8 contiguous ranges (rows c*12500..): core c
computes output rows for its range.  The full feature table is replicated per
core; layer-1 results are AllGather'd to rebuild the replicated table for
layer 2.

Aggregation (v2, tile-major): edges are grouped by dst tile (128 nodes), then
by 32768-row src window (int16 index limit of the dma_gather ucode).  All of
a tile's edge blocks accumulate into ONE PSUM tile (start on the first block,
stop on the last) — no SBUF aggregation buffer and no DVE adds.  For each
128-edge block a selection matrix sel[e, m] = (dstl[e] == m) * invdeg[dst[e]]
is built in one DVE op from a constant iota tile, and
  psum[f, m] += msgs[e, f]^T @ sel[e, m].
The transform emits node-major output directly:
  out[m, f_out] = aggT_t^T @ W_neigh + selfT_t^T @ W_self + 1^T @ bias
(bias via a 1-partition rank-1 matmul), then one fused ScalarE relu-copy and
a contiguous row write.  Layer-1 output rows are also PE-transposed in SBUF
to serve as layer-2's self features, which both skips a DRAM roundtrip and
lets them complete before the AllGather finishes.  PSUM->SBUF copies run on
the Scalar engine so the DVE queue only builds selection matrices.

The SPMD program is shared by all 8 cores, so per-(tile, pass) block counts
are static = max over the 8 cores; shorter cores pad with zero-weight slots.
"""

import numpy as np

N = 100000
F = 128
NCORES = 8
OWN = N // NCORES            # 12500
P = 128
NTILES = (OWN + P - 1) // P  # 98
OWN_PAD = NTILES * P         # 12544
N_PAD = 100096               # table rows padded to a multiple of 128
CHUNK = 32768
NPASS = (N + CHUNK - 1) // CHUNK  # 4
GBLK = 4                     # max 128-edge blocks per dma_gather instruction
SUPER = 1                    # dst tiles per supertile group
BLK = 128                    # edges per block


# --------------------------------------------------------------------------
# host-side planning
# --------------------------------------------------------------------------

def _plan(edge_src, edge_dst):
    src = np.asarray(edge_src).astype(np.int64).ravel()
    dst = np.asarray(edge_dst).astype(np.int64).ravel()
    deg = np.bincount(dst, minlength=N)
    invdeg = (1.0 / np.maximum(deg, 1)).astype(np.float32)

    per_core = []
    owner = dst // OWN
    for c in range(NCORES):
        m = owner == c
        s, d = src[m], dst[m]
        t = (d - c * OWN) // P
        p = s // CHUNK
        order = np.lexsort((p, t))
        per_core.append((s[order], d[order], t[order], p[order]))

    cnt = np.zeros((NCORES, NTILES, NPASS), dtype=np.int64)
    for c in range(NCORES):
        _s, _d, t, p = per_core[c]
        np.add.at(cnt, (c, t, p), 1)
    B = np.ceil(cnt.max(axis=0) / BLK).astype(np.int64)   # [NTILES, NPASS]

    nblk = int(B.sum())
    # block sequence: supertile groups of SUPER tiles; within a group,
    # pass-major across the group's tiles so same-window blocks are
    # consecutive and share large gather instructions.  Each group keeps
    # SUPER concurrent PSUM accumulation chains.
    order = []                     # (tile, pass) per block
    for g0 in range(0, NTILES, SUPER):
        gt_ = range(g0, min(g0 + SUPER, NTILES))
        for p in range(NPASS):
            for t in gt_:
                order.extend([(t, p)] * int(B[t, p]))
    blk_tile = np.array([t for t, _p in order], dtype=np.int64)
    blk_pass = np.array([p for _t, p in order], dtype=np.int64)
    assert len(order) == nblk

    # gather instructions: runs of consecutive same-pass blocks, <= GBLK
    insts = []                     # (pass, nblocks)
    i = 0
    while i < nblk:
        j = i
        while (j < nblk and j - i < GBLK and blk_pass[j] == blk_pass[i]):
            j += 1
        insts.append((int(blk_pass[i]), j - i))
        i = j

    # per-core packing in block order
    plans = []
    for c in range(NCORES):
        s, d, t, p = per_core[c]
        idx16 = np.zeros(nblk * BLK, dtype=np.int16)
        dstl = np.full(nblk * BLK, -1.0, dtype=np.float32)
        w = np.zeros(nblk * BLK, dtype=np.float32)
        # edges of (t, p) fill that cell's blocks, which are contiguous in
        # the order list
        cell0 = {}
        blk0 = 0
        for tt, pp in order:
            if (tt, pp) not in cell0:
                cell0[(tt, pp)] = blk0
            blk0 += 1
        for tt in range(NTILES):
            mt = t == tt
            st, dt_, pt = s[mt], d[mt], p[mt]
            for pp in range(NPASS):
                if B[tt, pp] == 0:
                    continue
                em = pt == pp
                se, de = st[em], dt_[em]
                ne = se.shape[0]
                assert ne <= int(B[tt, pp]) * BLK
                base = cell0[(tt, pp)] * BLK
                idx16[base : base + ne] = (se - pp * CHUNK).astype(np.int16)
                dstl[base : base + ne] = (de - c * OWN - tt * P).astype(
                    np.float32)
                w[base : base + ne] = invdeg[de]
        plans.append({"idx16": idx16, "dstl": dstl, "w": w})

    return plans, B, blk_tile, insts, nblk


def _pack_gidx(idx16, insts):
    """Pack int16 indices in the dma_gather SBUF layout (position j ->
    partition j%16, column j//16, replicated to 128 partitions) as one
    [128, total_cols] plane with per-instruction column segments, raveled
    partition-major.  Loaded to SBUF once and sliced per instruction."""
    total_cols = sum(take * BLK // 16 for _pp, take in insts)
    out = np.zeros((128, total_cols), dtype=np.int16)
    cursor = 0
    col = 0
    for _pp, take in insts:
        rows = take * BLK
        seg = idx16[cursor : cursor + rows]
        cursor += rows
        w16 = seg.reshape(rows // 16, 16).T          # [16, cols]
        out[:, col : col + rows // 16] = np.tile(w16, (8, 1))
        col += rows // 16
    return out.ravel()


# --------------------------------------------------------------------------
# device program
# --------------------------------------------------------------------------

def _build(B, blk_tile, insts, nblk, skip_collective=False, repeats=1,
           ablate=None, launder=False):
    import concourse.bass as bass
    import concourse.mybir as mybir
    import concourse.tile as tile
    from concourse import library_config
    from concourse.masks import make_identity
    from concourse.tile_rust import add_dep_helper

    nc = bass.Bass("TRN2", target_bir_lowering=False, debug=False,
                   num_devices=NCORES, num_swdge_queues=4)
    dt = mybir.dt

    x_rep = nc.dram_tensor("x_rep", [N_PAD, F], dt.float32,
                           kind="ExternalInput")
    x_self = nc.dram_tensor("x_self", [OWN_PAD, F], dt.float32,
                            kind="ExternalInput")
    gidx_len = sum(128 * (take * BLK // 16) for _pp, take in insts)
    gidx = nc.dram_tensor("gidx", [gidx_len], dt.int16, kind="ExternalInput")
    dstl_in = nc.dram_tensor("dstl", [P * nblk], dt.float32,
                             kind="ExternalInput")
    w_in = nc.dram_tensor("w", [P * nblk], dt.float32, kind="ExternalInput")
    iota_in = nc.dram_tensor("iota", [P * P], dt.float32,
                             kind="ExternalInput")
    ws1 = nc.dram_tensor("W_self1", [F, F], dt.float32, kind="ExternalInput")
    wn1 = nc.dram_tensor("W_neigh1", [F, F], dt.float32, kind="ExternalInput")
    b1 = nc.dram_tensor("b1", [F], dt.float32, kind="ExternalInput")
    ws2 = nc.dram_tensor("W_self2", [F, F], dt.float32, kind="ExternalInput")
    wn2 = nc.dram_tensor("W_neigh2", [F, F], dt.float32, kind="ExternalInput")
    b2 = nc.dram_tensor("b2", [F], dt.float32, kind="ExternalInput")
    out_shard = nc.dram_tensor("out_shard", [OWN_PAD, F], dt.float32,
                               kind="ExternalOutput")

    h1_own = nc.dram_tensor("h1_own", [OWN_PAD, F], dt.float32)
    h1_rep = nc.dram_tensor("h1_rep", [N_PAD, F], dt.float32,
                            addr_space="Shared")

    pass_len = [min(CHUNK, N - p * CHUNK) for p in range(NPASS)]

    # per-tile first/last block flags (tile blocks are noncontiguous within
    # a supertile group: pass-major across the group's tiles)
    first_idx = {}
    last_idx = {}
    for i in range(nblk):
        t = int(blk_tile[i])
        if t not in first_idx:
            first_idx[t] = i
        last_idx[t] = i
    firsts = [first_idx[int(blk_tile[i])] == i for i in range(nblk)]
    lasts = [last_idx[int(blk_tile[i])] == i for i in range(nblk)]

    with tile.TileContext(nc) as tc:
        with (
            tc.tile_pool(name="const", bufs=1) as cpool,
            tc.tile_pool(name="selfbig", bufs=1) as sbpool,
            tc.tile_pool(name="gather", bufs=12) as gpool,
            tc.tile_pool(name="sel", bufs=8) as spool,
            tc.tile_pool(name="stage", bufs=5) as stpool,
            tc.tile_pool(name="psA", bufs=4, space="PSUM") as ppoolA,
            tc.tile_pool(name="psB", bufs=2, space="PSUM") as ppoolB,
        ):
            lib = nc.gpsimd.load_library(library_config.mlp)
            rows_regs = {}

            def rows_reg(v):
                if v not in rows_regs:
                    rows_regs[v] = nc.gpsimd.to_reg(v)
                return rows_regs[v]

            iota = cpool.tile([P, P], dt.float32)
            nc.sync.dma_start(out=iota[:],
                              in_=iota_in.ap().rearrange("(p f) -> p f", p=P))
            ident = cpool.tile([P, P], dt.float32)
            make_identity(nc, ident[:])
            ones1 = cpool.tile([1, P], dt.float32)
            nc.vector.memset(ones1[:], 1.0)

            wtiles = {}
            for name, t in (("ws1", ws1), ("wn1", wn1), ("ws2", ws2),
                            ("wn2", wn2)):
                wt = cpool.tile([P, P], dt.float32, name=f"w_{name}",
                                tag=f"w_{name}")
                nc.sync.dma_start(out=wt[:], in_=t[:, :])
                wtiles[name] = wt
            btiles = {}
            for name, t in (("b1", b1), ("b2", b2)):
                bt = cpool.tile([1, P], dt.float32, name=f"b_{name}",
                                tag=f"b_{name}")
                nc.sync.dma_start(out=bt[:], in_=t.ap()[None, :])
                btiles[name] = bt

            gidx_t = cpool.tile([P, gidx_len // P], dt.int16)
            nc.sync.dma_start(out=gidx_t[:],
                              in_=gidx.ap().rearrange("(p k) -> p k", p=P))
            dstl_t = cpool.tile([P, nblk], dt.float32)
            nc.sync.dma_start(out=dstl_t[:],
                              in_=dstl_in.ap().rearrange("(p b) -> p b", p=P))
            w_t = cpool.tile([P, nblk], dt.float32)
            nc.sync.dma_start(out=w_t[:],
                              in_=w_in.ap().rearrange("(p b) -> p b", p=P))

            # device loop mirrors of the static schedule
            # per tile: list of (inst_index, take); inst i covers blocks
            # [inst_blk0[i], inst_blk0[i]+take)
            inst_blk0 = []
            bcur = 0
            for _pp, take in insts:
                inst_blk0.append(bcur)
                bcur += take
            assert bcur == nblk

            def run_layer(layer, table, self_big, wself, wneigh, bias,
                          dest, next_self):
                """next_self: SBUF big tile to fill with dest^T, or None."""
                # layer 1: self features transposed per-tile from DRAM x_self
                # into a small rotating ring (layer 2 reads the big SBUF
                # buffer filled during layer 1 instead)
                selft = {}
                if self_big is None:
                    def self_tile(t):
                        xt = stpool.tile([P, P], dt.float32, tag="xt")
                        nc.sync.dma_start(
                            out=xt[:], in_=x_self[t * P : (t + 1) * P, :])
                        pst = ppoolB.tile([P, P], dt.float32, tag="pst",
                                          space="PSUM")
                        nc.tensor.transpose(out=pst[:], in_=xt[:],
                                            identity=ident[:])
                        st = stpool.tile([P, P], dt.float32, tag="selft")
                        nc.scalar.activation(
                            st[:], pst[:],
                            mybir.ActivationFunctionType.Copy)
                        return st[:]
                else:
                    def self_tile(t):
                        return self_big[:, t * P : (t + 1) * P]

                live_psum = {}
                writes = []
                for i, (pp, take) in enumerate(insts):
                    b0 = inst_blk0[i]
                    gt = gpool.tile([P, GBLK * P], dt.float32, tag="g")
                    if ablate != "no_gather":
                        g = nc.gpsimd.dma_gather(
                            gt[:, : take * P].rearrange("p (b f) -> p b f",
                                                        f=P),
                            table[pp * CHUNK : pp * CHUNK + pass_len[pp], :],
                            gidx_t[:, b0 * BLK // 16 : (b0 + take) * BLK // 16],
                            take * BLK,
                            rows_reg(take * BLK),
                            F,
                            queue_num=i % 4,
                        )
                        add_dep_helper(g.ins, lib.ins, sync=False,
                                       reason="ucode lib before gather")
                    else:
                        nc.sync.dma_start(
                            out=gt[:, : take * P].rearrange(
                                "p (b f) -> p b f", f=P),
                            in_=table[0 : take * BLK, :].rearrange(
                                "(b p) f -> p b f", p=P),
                        )
                    if ablate == "gather_only":
                        probe = spool.tile([P, GBLK * P], dt.float32,
                                           tag="probe")
                        nc.vector.tensor_copy(out=probe[:, : take * P],
                                              in_=gt[:, : take * P])
                        continue
                    if launder:
                        gt2 = gpool.tile([P, GBLK * P], dt.float32, tag="g2")
                        nc.scalar.activation(
                            gt2[:, : take * P], gt[:, : take * P],
                            mybir.ActivationFunctionType.Copy)
                        gt = gt2

                    for k in range(take):
                        b = b0 + k
                        t = int(blk_tile[b])
                        if ablate == "fixed_sel":
                            sel = iota
                        else:
                            sel = spool.tile([P, P], dt.float32, tag="sel")
                            nc.vector.tensor_scalar(
                                sel[:], iota[:],
                                dstl_t[:, b : b + 1], w_t[:, b : b + 1],
                                mybir.AluOpType.is_equal,
                                mybir.AluOpType.mult,
                            )
                        if firsts[b]:
                            live_psum[t] = ppoolA.tile([P, P], dt.float32,
                                                       name="ps", tag="ps",
                                                       space="PSUM")
                            selft[t] = self_tile(t)
                        ps = live_psum[t]
                        nc.tensor.matmul(
                            out=ps[:], lhsT=gt[:, k * P : (k + 1) * P],
                            rhs=sel[:],
                            start=bool(firsts[b]), stop=bool(lasts[b]),
                        )
                        if lasts[b]:
                            del live_psum[t]
                            if ablate == "no_transform":
                                pr = stpool.tile([P, 1], dt.float32,
                                                 tag="pprobe")
                                nc.scalar.activation(
                                    pr[:], ps[:, :1],
                                    mybir.ActivationFunctionType.Copy)
                                continue
                            aggt = stpool.tile([P, P], dt.float32, tag="aggt")
                            nc.scalar.activation(
                                aggt[:], ps[:],
                                mybir.ActivationFunctionType.Copy)
                            # transform: node-major out tile
                            oc = ppoolB.tile([P, P], dt.float32, tag="oc",
                                             space="PSUM")
                            nc.tensor.matmul(out=oc[:], lhsT=aggt[:],
                                             rhs=wneigh[:],
                                             start=True, stop=False)
                            nc.tensor.matmul(
                                out=oc[:],
                                lhsT=selft.pop(t),
                                rhs=wself[:], start=False, stop=False)
                            nc.tensor.matmul(out=oc[:], lhsT=ones1[:],
                                             rhs=bias[:],
                                             start=False, stop=True)
                            ot = stpool.tile([P, P], dt.float32, tag="ot")
                            nc.scalar.activation(
                                ot[:], oc[:],
                                mybir.ActivationFunctionType.Relu)
                            if next_self is not None:
                                pso = ppoolB.tile([P, P], dt.float32,
                                                  tag="pst", space="PSUM")
                                nc.tensor.transpose(out=pso[:], in_=ot[:],
                                                    identity=ident[:])
                                nc.scalar.activation(
                                    next_self[:, t * P : (t + 1) * P],
                                    pso[:],
                                    mybir.ActivationFunctionType.Copy)
                            dd = nc.sync.dma_start(
                                out=dest[t * P : (t + 1) * P, :],
                                in_=ot[:],
                            )
                            writes.append(dd)
                return writes

            for _rep in range(repeats):
                self2 = sbpool.tile([P, NTILES * P], dt.float32, tag="selfb")
                run_layer(0, x_rep, None, wtiles["ws1"], wtiles["wn1"],
                          btiles["b1"], h1_own, self2)

                if ablate == "gather_only":
                    continue
                if skip_collective:
                    nc.sync.dma_start(out=h1_rep[0:OWN, :],
                                      in_=h1_own[0:OWN, :])
                else:
                    nc.gpsimd.collective_compute(
                        "AllGather",
                        mybir.AluOpType.bypass,
                        replica_groups=[list(range(NCORES))],
                        ins=[h1_own[0:OWN, :]],
                        outs=[h1_rep[0:N, :]],
                    )
                if N_PAD > N:
                    zt = stpool.tile([P, F], dt.float32, tag="zt")
                    nc.vector.memset(zt[:], 0.0)
                    nc.sync.dma_start(out=h1_rep[N:N_PAD, :],
                                      in_=zt[: N_PAD - N, :])

                run_layer(1, h1_rep, self2, wtiles["ws2"], wtiles["wn2"],
                          btiles["b2"], out_shard, None)

    _split_multi_waits(nc)
    from concourse.library_overlay import lower_extended_insts
    lower_extended_insts(nc)
    return nc


def _split_multi_waits(nc):
    """Walrus codegen encodes at most one sync wait per instruction; split
    extras into standalone EventSemaphore instructions on the same in-order
    engine queue (semantically identical)."""
    import concourse.mybir as mybir

    n = 0
    for f in nc.m.functions:
        for b in f.blocks:
            insts = b.instructions
            new_list = []
            for inst in insts:
                si = inst.sync_info
                if si is not None and len(si.on_wait) > 1:
                    waits = list(si.on_wait)
                    for wt in waits[:-1]:
                        ev = mybir.InstEventSemaphore(
                            name=f"evsplit-{n}",
                            engine=inst.engine,
                            sync_info=mybir.SyncInfo(on_wait=[wt],
                                                     on_update=[]),
                            ins=[], outs=[],
                        )
                        new_list.append(ev)
                        try:
                            nc.inst_map[ev.name] = ev
                        except Exception:
                            pass
                        n += 1
                    inst.sync_info = mybir.SyncInfo(
                        on_wait=[waits[-1]], on_update=list(si.on_update)
                    )
                new_list.append(inst)
            insts[:] = new_list
    return n


# --------------------------------------------------------------------------
# entry point
# --------------------------------------------------------------------------

def prepare(x, edge_src, edge_dst, W_self1, W_neigh1, b1, W_self2, W_neigh2,
            b2):
    """Plan + build + pack inputs; returns (nc, in_maps, finish)."""
    x = np.asarray(x, dtype=np.float32)
    plans, B, blk_tile, insts, nblk = _plan(edge_src, edge_dst)

    xpad = np.zeros((N_PAD, F), dtype=np.float32)
    xpad[:N] = x
    iota = np.broadcast_to(np.arange(P, dtype=np.float32), (P, P))

    in_maps = []
    for c in range(NCORES):
        pl = plans[c]
        xs = np.zeros((OWN_PAD, F), dtype=np.float32)
        xs[:OWN] = x[c * OWN : (c + 1) * OWN]
        in_maps.append({
            "x_rep": xpad,
            "x_self": xs,
            "gidx": _pack_gidx(pl["idx16"], insts),
            "dstl": pl["dstl"].reshape(nblk, P).T.copy().ravel(),
            "w": pl["w"].reshape(nblk, P).T.copy().ravel(),
            "iota": np.ascontiguousarray(iota).ravel(),
            "W_self1": np.asarray(W_self1, np.float32),
            "W_neigh1": np.asarray(W_neigh1, np.float32),
            "b1": np.asarray(b1, np.float32),
            "W_self2": np.asarray(W_self2, np.float32),
            "W_neigh2": np.asarray(W_neigh2, np.float32),
            "b2": np.asarray(b2, np.float32),
        })

    nc = _build(B, blk_tile, insts, nblk)

    def finish(results):
        return np.concatenate(
            [results[c]["out_shard"][:OWN] for c in range(NCORES)], axis=0
        ).astype(np.float32)

    return nc, in_maps, finish


def kernel(x, edge_src, edge_dst, W_self1, W_neigh1, b1, W_self2, W_neigh2,
           b2, trace=False, _return_res=False):
    from concourse.bass_utils import run_bass_kernel_spmd

    nc, in_maps, finish = prepare(x, edge_src, edge_dst, W_self1, W_neigh1,
                                  b1, W_self2, W_neigh2, b2)
    res = run_bass_kernel_spmd(nc, in_maps, list(range(NCORES)), trace=trace)
    out = finish(res.results)
    if _return_res:
        return out, res
    return out


# revision 37
# speedup vs baseline: 48.3874x; 1.4944x over previous
"""GraphSAGE-mean 2-layer GNN on 8 Trainium2 NeuronCores (Bass/Tile).

Sharding: nodes split into 8 contiguous ranges (rows c*12500..): core c
computes output rows for its range.  The full feature table is replicated per
core; layer-1 results are AllGather'd to rebuild the replicated table for
layer 2.

Aggregation (v2, tile-major): edges are grouped by dst tile (128 nodes), then
by 32768-row src window (int16 index limit of the dma_gather ucode).  All of
a tile's edge blocks accumulate into ONE PSUM tile (start on the first block,
stop on the last) — no SBUF aggregation buffer and no DVE adds.  For each
128-edge block a selection matrix sel[e, m] = (dstl[e] == m) * invdeg[dst[e]]
is built in one DVE op from a constant iota tile, and
  psum[f, m] += msgs[e, f]^T @ sel[e, m].
The transform emits node-major output directly:
  out[m, f_out] = aggT_t^T @ W_neigh + selfT_t^T @ W_self + 1^T @ bias
(bias via a 1-partition rank-1 matmul), then one fused ScalarE relu-copy and
a contiguous row write.  Layer-1 output rows are also PE-transposed in SBUF
to serve as layer-2's self features, which both skips a DRAM roundtrip and
lets them complete before the AllGather finishes.  PSUM->SBUF copies run on
the Scalar engine so the DVE queue only builds selection matrices.

The SPMD program is shared by all 8 cores, so per-(tile, pass) block counts
are static = max over the 8 cores; shorter cores pad with zero-weight slots.
"""

import numpy as np

N = 100000
F = 128
NCORES = 8
OWN = N // NCORES            # 12500
P = 128
NTILES = (OWN + P - 1) // P  # 98
OWN_PAD = NTILES * P         # 12544
N_PAD = 100096               # table rows padded to a multiple of 128
CHUNK = 32768
NPASS = (N + CHUNK - 1) // CHUNK  # 4
GBLK = 4                     # max 128-edge blocks per dma_gather instruction
SUPER = 1                    # dst tiles per supertile group
BLK = 128                    # edges per block


# --------------------------------------------------------------------------
# host-side planning
# --------------------------------------------------------------------------

def _plan(edge_src, edge_dst):
    src = np.asarray(edge_src).astype(np.int64).ravel()
    dst = np.asarray(edge_dst).astype(np.int64).ravel()
    deg = np.bincount(dst, minlength=N)
    invdeg = (1.0 / np.maximum(deg, 1)).astype(np.float32)

    per_core = []
    owner = dst // OWN
    for c in range(NCORES):
        m = owner == c
        s, d = src[m], dst[m]
        t = (d - c * OWN) // P
        p = s // CHUNK
        order = np.lexsort((p, t))
        per_core.append((s[order], d[order], t[order], p[order]))

    cnt = np.zeros((NCORES, NTILES, NPASS), dtype=np.int64)
    for c in range(NCORES):
        _s, _d, t, p = per_core[c]
        np.add.at(cnt, (c, t, p), 1)
    B = np.ceil(cnt.max(axis=0) / BLK).astype(np.int64)   # [NTILES, NPASS]

    nblk = int(B.sum())
    # block sequence: supertile groups of SUPER tiles; within a group,
    # pass-major across the group's tiles so same-window blocks are
    # consecutive and share large gather instructions.  Each group keeps
    # SUPER concurrent PSUM accumulation chains.
    order = []                     # (tile, pass) per block
    for g0 in range(0, NTILES, SUPER):
        gt_ = range(g0, min(g0 + SUPER, NTILES))
        for p in range(NPASS):
            for t in gt_:
                order.extend([(t, p)] * int(B[t, p]))
    blk_tile = np.array([t for t, _p in order], dtype=np.int64)
    blk_pass = np.array([p for _t, p in order], dtype=np.int64)
    assert len(order) == nblk

    # gather instructions: runs of consecutive same-pass blocks, <= GBLK
    insts = []                     # (pass, nblocks)
    i = 0
    while i < nblk:
        j = i
        while (j < nblk and j - i < GBLK and blk_pass[j] == blk_pass[i]):
            j += 1
        insts.append((int(blk_pass[i]), j - i))
        i = j

    # per-core packing in block order
    plans = []
    for c in range(NCORES):
        s, d, t, p = per_core[c]
        idx16 = np.zeros(nblk * BLK, dtype=np.int16)
        dstl = np.full(nblk * BLK, -1.0, dtype=np.float32)
        w = np.zeros(nblk * BLK, dtype=np.float32)
        # edges of (t, p) fill that cell's blocks, which are contiguous in
        # the order list
        cell0 = {}
        blk0 = 0
        for tt, pp in order:
            if (tt, pp) not in cell0:
                cell0[(tt, pp)] = blk0
            blk0 += 1
        for tt in range(NTILES):
            mt = t == tt
            st, dt_, pt = s[mt], d[mt], p[mt]
            for pp in range(NPASS):
                if B[tt, pp] == 0:
                    continue
                em = pt == pp
                se, de = st[em], dt_[em]
                ne = se.shape[0]
                assert ne <= int(B[tt, pp]) * BLK
                base = cell0[(tt, pp)] * BLK
                idx16[base : base + ne] = (se - pp * CHUNK).astype(np.int16)
                dstl[base : base + ne] = (de - c * OWN - tt * P).astype(
                    np.float32)
                w[base : base + ne] = invdeg[de]
        plans.append({"idx16": idx16, "dstl": dstl, "w": w})

    return plans, B, blk_tile, insts, nblk


def _pack_gidx(idx16, insts):
    """Pack int16 indices in the dma_gather SBUF layout (position j ->
    partition j%16, column j//16, replicated to 128 partitions) as one
    [128, total_cols] plane with per-instruction column segments, raveled
    partition-major.  Loaded to SBUF once and sliced per instruction."""
    total_cols = sum(take * BLK // 16 for _pp, take in insts)
    out = np.zeros((128, total_cols), dtype=np.int16)
    cursor = 0
    col = 0
    for _pp, take in insts:
        rows = take * BLK
        seg = idx16[cursor : cursor + rows]
        cursor += rows
        w16 = seg.reshape(rows // 16, 16).T          # [16, cols]
        out[:, col : col + rows // 16] = np.tile(w16, (8, 1))
        col += rows // 16
    return out.ravel()


# --------------------------------------------------------------------------
# device program
# --------------------------------------------------------------------------

def _build(B, blk_tile, insts, nblk, skip_collective=False, repeats=1,
           ablate=None, launder=False):
    import concourse.bass as bass
    import concourse.mybir as mybir
    import concourse.tile as tile
    from concourse import library_config
    from concourse.masks import make_identity
    from concourse.tile_rust import add_dep_helper

    nc = bass.Bass("TRN2", target_bir_lowering=False, debug=False,
                   num_devices=NCORES, num_swdge_queues=4)
    dt = mybir.dt

    x_rep = nc.dram_tensor("x_rep", [N_PAD, F], dt.bfloat16,
                           kind="ExternalInput")
    x_self = nc.dram_tensor("x_self", [OWN_PAD, F], dt.float32,
                            kind="ExternalInput")
    gidx_len = sum(128 * (take * BLK // 16) for _pp, take in insts)
    gidx = nc.dram_tensor("gidx", [gidx_len], dt.int16, kind="ExternalInput")
    dstl_in = nc.dram_tensor("dstl", [P * nblk], dt.float32,
                             kind="ExternalInput")
    w_in = nc.dram_tensor("w", [P * nblk], dt.float32,
                          kind="ExternalInput")
    iota_in = nc.dram_tensor("iota", [P * P], dt.bfloat16,
                             kind="ExternalInput")
    ws1 = nc.dram_tensor("W_self1", [F, F], dt.float32, kind="ExternalInput")
    wn1 = nc.dram_tensor("W_neigh1", [F, F], dt.float32, kind="ExternalInput")
    b1 = nc.dram_tensor("b1", [F], dt.float32, kind="ExternalInput")
    ws2 = nc.dram_tensor("W_self2", [F, F], dt.float32, kind="ExternalInput")
    wn2 = nc.dram_tensor("W_neigh2", [F, F], dt.float32, kind="ExternalInput")
    b2 = nc.dram_tensor("b2", [F], dt.float32, kind="ExternalInput")
    out_shard = nc.dram_tensor("out_shard", [OWN_PAD, F], dt.float32,
                               kind="ExternalOutput")

    h1_own = nc.dram_tensor("h1_own", [OWN_PAD, F], dt.bfloat16)
    h1_rep = nc.dram_tensor("h1_rep", [N_PAD, F], dt.bfloat16,
                            addr_space="Shared")

    pass_len = [min(CHUNK, N - p * CHUNK) for p in range(NPASS)]

    # per-tile first/last block flags (tile blocks are noncontiguous within
    # a supertile group: pass-major across the group's tiles)
    first_idx = {}
    last_idx = {}
    for i in range(nblk):
        t = int(blk_tile[i])
        if t not in first_idx:
            first_idx[t] = i
        last_idx[t] = i
    firsts = [first_idx[int(blk_tile[i])] == i for i in range(nblk)]
    lasts = [last_idx[int(blk_tile[i])] == i for i in range(nblk)]

    with tile.TileContext(nc) as tc:
        with (
            tc.tile_pool(name="const", bufs=1) as cpool,
            tc.tile_pool(name="selfbig", bufs=1) as sbpool,
            tc.tile_pool(name="gather", bufs=12) as gpool,
            tc.tile_pool(name="sel", bufs=8) as spool,
            tc.tile_pool(name="stage", bufs=5) as stpool,
            tc.tile_pool(name="psA", bufs=4, space="PSUM") as ppoolA,
            tc.tile_pool(name="psB", bufs=2, space="PSUM") as ppoolB,
        ):
            lib = nc.gpsimd.load_library(library_config.mlp)
            rows_regs = {}

            def rows_reg(v):
                if v not in rows_regs:
                    rows_regs[v] = nc.gpsimd.to_reg(v)
                return rows_regs[v]

            iota = cpool.tile([P, P], dt.bfloat16)
            nc.sync.dma_start(out=iota[:],
                              in_=iota_in.ap().rearrange("(p f) -> p f", p=P))
            ident = cpool.tile([P, P], dt.float32)
            make_identity(nc, ident[:])
            ident_bf = cpool.tile([P, P], dt.bfloat16)
            nc.vector.tensor_copy(out=ident_bf[:], in_=ident[:])
            ones1 = cpool.tile([1, P], dt.float32)
            nc.vector.memset(ones1[:], 1.0)

            wtiles = {}
            for name, t in (("ws1", ws1), ("wn1", wn1), ("ws2", ws2),
                            ("wn2", wn2)):
                wt = cpool.tile([P, P], dt.float32, name=f"w_{name}",
                                tag=f"w_{name}")
                nc.sync.dma_start(out=wt[:], in_=t[:, :])
                wtiles[name] = wt
            btiles = {}
            for name, t in (("b1", b1), ("b2", b2)):
                bt = cpool.tile([1, P], dt.float32, name=f"b_{name}",
                                tag=f"b_{name}")
                nc.sync.dma_start(out=bt[:], in_=t.ap()[None, :])
                btiles[name] = bt

            gidx_t = cpool.tile([P, gidx_len // P], dt.int16)
            nc.sync.dma_start(out=gidx_t[:],
                              in_=gidx.ap().rearrange("(p k) -> p k", p=P))
            dstl_t = cpool.tile([P, nblk], dt.float32)
            nc.sync.dma_start(out=dstl_t[:],
                              in_=dstl_in.ap().rearrange("(p b) -> p b", p=P))
            w_t = cpool.tile([P, nblk], dt.float32)
            nc.sync.dma_start(out=w_t[:],
                              in_=w_in.ap().rearrange("(p b) -> p b", p=P))

            # device loop mirrors of the static schedule
            # per tile: list of (inst_index, take); inst i covers blocks
            # [inst_blk0[i], inst_blk0[i]+take)
            inst_blk0 = []
            bcur = 0
            for _pp, take in insts:
                inst_blk0.append(bcur)
                bcur += take
            assert bcur == nblk

            def run_layer(layer, table, self_big, wself, wneigh, bias,
                          dest, next_self, ddt):
                """next_self: SBUF big tile to fill with dest^T, or None."""
                # layer 1: self features transposed per-tile from DRAM x_self
                # into a small rotating ring (layer 2 reads the big SBUF
                # buffer filled during layer 1 instead)
                selft = {}
                if self_big is None:
                    def self_tile(t):
                        xt = stpool.tile([P, P], dt.float32, tag="xt")
                        nc.sync.dma_start(
                            out=xt[:], in_=x_self[t * P : (t + 1) * P, :])
                        pst = ppoolB.tile([P, P], dt.float32, tag="pst",
                                          space="PSUM")
                        nc.tensor.transpose(out=pst[:], in_=xt[:],
                                            identity=ident[:])
                        st = stpool.tile([P, P], dt.float32, tag="selft")
                        nc.scalar.activation(
                            st[:], pst[:],
                            mybir.ActivationFunctionType.Copy)
                        return st[:]
                else:
                    def self_tile(t):
                        return self_big[:, t * P : (t + 1) * P]

                live_psum = {}
                writes = []
                for i, (pp, take) in enumerate(insts):
                    b0 = inst_blk0[i]
                    gt = gpool.tile([P, GBLK * P], dt.bfloat16, tag="g")
                    if ablate != "no_gather":
                        g = nc.gpsimd.dma_gather(
                            gt[:, : take * P].rearrange("p (b f) -> p b f",
                                                        f=P),
                            table[pp * CHUNK : pp * CHUNK + pass_len[pp], :],
                            gidx_t[:, b0 * BLK // 16 : (b0 + take) * BLK // 16],
                            take * BLK,
                            rows_reg(take * BLK),
                            F,
                            queue_num=i % 4,
                        )
                        add_dep_helper(g.ins, lib.ins, sync=False,
                                       reason="ucode lib before gather")
                    else:
                        nc.sync.dma_start(
                            out=gt[:, : take * P].rearrange(
                                "p (b f) -> p b f", f=P),
                            in_=table[0 : take * BLK, :].rearrange(
                                "(b p) f -> p b f", p=P),
                        )
                    if ablate == "gather_only":
                        probe = spool.tile([P, GBLK * P], dt.bfloat16,
                                           tag="probe")
                        nc.vector.tensor_copy(out=probe[:, : take * P],
                                              in_=gt[:, : take * P])
                        continue
                    if launder:
                        gt2 = gpool.tile([P, GBLK * P], dt.bfloat16, tag="g2")
                        nc.scalar.activation(
                            gt2[:, : take * P], gt[:, : take * P],
                            mybir.ActivationFunctionType.Copy)
                        gt = gt2

                    for k in range(take):
                        b = b0 + k
                        t = int(blk_tile[b])
                        if ablate == "fixed_sel":
                            sel = iota
                        else:
                            sel = spool.tile([P, P], dt.bfloat16, tag="sel")
                            nc.vector.tensor_scalar(
                                sel[:], iota[:],
                                dstl_t[:, b : b + 1], w_t[:, b : b + 1],
                                mybir.AluOpType.is_equal,
                                mybir.AluOpType.mult,
                            )
                        if firsts[b]:
                            live_psum[t] = ppoolA.tile([P, P], dt.float32,
                                                       name="ps", tag="ps",
                                                       space="PSUM")
                            selft[t] = self_tile(t)
                        ps = live_psum[t]
                        nc.tensor.matmul(
                            out=ps[:], lhsT=gt[:, k * P : (k + 1) * P],
                            rhs=sel[:],
                            start=bool(firsts[b]), stop=bool(lasts[b]),
                        )
                        if lasts[b]:
                            del live_psum[t]
                            if ablate == "no_transform":
                                pr = stpool.tile([P, 1], dt.float32,
                                                 tag="pprobe")
                                nc.scalar.activation(
                                    pr[:], ps[:, :1],
                                    mybir.ActivationFunctionType.Copy)
                                continue
                            aggt = stpool.tile([P, P], dt.float32, tag="aggt")
                            nc.scalar.activation(
                                aggt[:], ps[:],
                                mybir.ActivationFunctionType.Copy)
                            # transform: node-major out tile
                            oc = ppoolB.tile([P, P], dt.float32, tag="oc",
                                             space="PSUM")
                            nc.tensor.matmul(out=oc[:], lhsT=aggt[:],
                                             rhs=wneigh[:],
                                             start=True, stop=False)
                            nc.tensor.matmul(
                                out=oc[:],
                                lhsT=selft.pop(t),
                                rhs=wself[:], start=False, stop=False)
                            nc.tensor.matmul(out=oc[:], lhsT=ones1[:],
                                             rhs=bias[:],
                                             start=False, stop=True)
                            ot = stpool.tile([P, P], ddt, tag="ot")
                            nc.scalar.activation(
                                ot[:], oc[:],
                                mybir.ActivationFunctionType.Relu)
                            if next_self is not None:
                                pso = ppoolB.tile([P, P], ddt,
                                                  name="pso", tag="pst",
                                                  space="PSUM")
                                nc.tensor.transpose(
                                    out=pso[:], in_=ot[:],
                                    identity=ident_bf[:]
                                    if ddt == dt.bfloat16 else ident[:])
                                nc.scalar.activation(
                                    next_self[:, t * P : (t + 1) * P],
                                    pso[:],
                                    mybir.ActivationFunctionType.Copy)
                            dd = nc.sync.dma_start(
                                out=dest[t * P : (t + 1) * P, :],
                                in_=ot[:],
                            )
                            writes.append(dd)
                return writes

            for _rep in range(repeats):
                self2 = sbpool.tile([P, NTILES * P], dt.float32, tag="selfb")
                run_layer(0, x_rep, None, wtiles["ws1"], wtiles["wn1"],
                          btiles["b1"], h1_own, self2, dt.bfloat16)

                if ablate == "gather_only":
                    continue
                if skip_collective:
                    nc.sync.dma_start(out=h1_rep[0:OWN, :],
                                      in_=h1_own[0:OWN, :])
                else:
                    nc.gpsimd.collective_compute(
                        "AllGather",
                        mybir.AluOpType.bypass,
                        replica_groups=[list(range(NCORES))],
                        ins=[h1_own[0:OWN, :]],
                        outs=[h1_rep[0:N, :]],
                    )
                if N_PAD > N:
                    zt = stpool.tile([P, F], dt.bfloat16, tag="zt")
                    nc.vector.memset(zt[:], 0.0)
                    nc.sync.dma_start(out=h1_rep[N:N_PAD, :],
                                      in_=zt[: N_PAD - N, :])

                run_layer(1, h1_rep, self2, wtiles["ws2"], wtiles["wn2"],
                          btiles["b2"], out_shard, None, dt.float32)

    _split_multi_waits(nc)
    from concourse.library_overlay import lower_extended_insts
    lower_extended_insts(nc)
    return nc


def _split_multi_waits(nc):
    """Walrus codegen encodes at most one sync wait per instruction; split
    extras into standalone EventSemaphore instructions on the same in-order
    engine queue (semantically identical)."""
    import concourse.mybir as mybir

    n = 0
    for f in nc.m.functions:
        for b in f.blocks:
            insts = b.instructions
            new_list = []
            for inst in insts:
                si = inst.sync_info
                if si is not None and len(si.on_wait) > 1:
                    waits = list(si.on_wait)
                    for wt in waits[:-1]:
                        ev = mybir.InstEventSemaphore(
                            name=f"evsplit-{n}",
                            engine=inst.engine,
                            sync_info=mybir.SyncInfo(on_wait=[wt],
                                                     on_update=[]),
                            ins=[], outs=[],
                        )
                        new_list.append(ev)
                        try:
                            nc.inst_map[ev.name] = ev
                        except Exception:
                            pass
                        n += 1
                    inst.sync_info = mybir.SyncInfo(
                        on_wait=[waits[-1]], on_update=list(si.on_update)
                    )
                new_list.append(inst)
            insts[:] = new_list
    return n


# --------------------------------------------------------------------------
# entry point
# --------------------------------------------------------------------------

def prepare(x, edge_src, edge_dst, W_self1, W_neigh1, b1, W_self2, W_neigh2,
            b2):
    """Plan + build + pack inputs; returns (nc, in_maps, finish)."""
    x = np.asarray(x, dtype=np.float32)
    plans, B, blk_tile, insts, nblk = _plan(edge_src, edge_dst)

    import ml_dtypes
    xpad = np.zeros((N_PAD, F), dtype=np.float32)
    xpad[:N] = x
    xpad = xpad.astype(ml_dtypes.bfloat16)
    iota = np.broadcast_to(np.arange(P, dtype=np.float32),
                           (P, P)).astype(ml_dtypes.bfloat16)

    in_maps = []
    for c in range(NCORES):
        pl = plans[c]
        xs = np.zeros((OWN_PAD, F), dtype=np.float32)
        xs[:OWN] = x[c * OWN : (c + 1) * OWN]
        in_maps.append({
            "x_rep": xpad,
            "x_self": xs,
            "gidx": _pack_gidx(pl["idx16"], insts),
            "dstl": pl["dstl"].reshape(nblk, P).T.copy().ravel(),
            "w": pl["w"].reshape(nblk, P).T.copy().ravel(),
            "iota": np.ascontiguousarray(iota).ravel(),
            "W_self1": np.asarray(W_self1, np.float32),
            "W_neigh1": np.asarray(W_neigh1, np.float32),
            "b1": np.asarray(b1, np.float32),
            "W_self2": np.asarray(W_self2, np.float32),
            "W_neigh2": np.asarray(W_neigh2, np.float32),
            "b2": np.asarray(b2, np.float32),
        })

    nc = _build(B, blk_tile, insts, nblk)

    def finish(results):
        return np.concatenate(
            [results[c]["out_shard"][:OWN] for c in range(NCORES)], axis=0
        ).astype(np.float32)

    return nc, in_maps, finish


def kernel(x, edge_src, edge_dst, W_self1, W_neigh1, b1, W_self2, W_neigh2,
           b2, trace=False, _return_res=False):
    from concourse.bass_utils import run_bass_kernel_spmd

    nc, in_maps, finish = prepare(x, edge_src, edge_dst, W_self1, W_neigh1,
                                  b1, W_self2, W_neigh2, b2)
    res = run_bass_kernel_spmd(nc, in_maps, list(range(NCORES)), trace=trace)
    out = finish(res.results)
    if _return_res:
        return out, res
    return out
